# revision 57
# baseline (speedup 1.0000x reference)
"""Trainium2 Bass kernel for grouped-query attention with qk-norm.

Problem (hardcoded): x(2,2048,1024) @ Wq(1024,1024) / Wkv(1024,512),
16 query heads, 4 kv heads, head_dim 64, k_scale(16,1,64) applied to the
group-broadcast k. Output (2,2048,1024).

Sharding: 8 cores = batch(2) x kv_heads(4). Each core computes its batch's
4 query heads against its kv head over the full 2048x2048 score matrix.

Device kernel layout choices:
- Host passes x transposed and kt-tiled (dim on partitions) and all
  weights pre-tiled to [128, kt-major], so every input DMA moves
  contiguous 1-8KB lines and projections need no on-device transposes.
- k_scale is folded into Wq host-side ((q*ks)@k^T == q@(k*ks)^T); k is
  projected once per kv head with a host-duplicated [wk|wk] stationary
  (M=128 costs the same as M=64) so PSUM holds kT on both partition
  halves and one copy fills both QK row-tile stationary slots.
- Scores are computed transposed (S^T: keys on partitions, queries free)
  so exp(S^T) feeds the PV matmul directly as the moving operand. All
  matmul inputs are fp16 (fp32r streams at half rate).
- Softmax skips the max-subtraction (inputs bounded) and normalizes
  after PV via an appended ones-row in the V stationary (row 64 of the
  PV psum accumulates sum(exp)).
- The ScalarE exp stream is the pipeline limiter (~1ns/col + 352-cycle
  per-instruction overhead), so: the ramp runs BOTH hp-blocks of the
  first query chunk against each key chunk as it is projected (8 exps
  per projected chunk, PV group-accumulated into SBUF to fit PSUM), and
  the steady phase batches exp into 1536-wide ACTIVATEs (3 jt per 2
  ACTs) with PV accumulating in PSUM.
- Output is returned transposed per head (oT: 4*64 x 2048); the host
  transposes during the gather.
"""

from contextlib import ExitStack

import numpy as np

import concourse.bacc as bacc
import concourse.mybir as mybir
import concourse.tile as tile
from concourse.bass_utils import run_bass_kernel_spmd

# Problem constants
B, N, DIM = 2, 2048, 1024
HEADS, KV_HEADS, DH = 16, 4, 64
G = HEADS // KV_HEADS  # query heads per kv head (4)
NCORES = 8
P = 128
KT = DIM // P  # 8 contraction tiles over dim
IC = 512  # query-chunk width
NI = N // IC  # 4
NJ = N // P  # 16 key tiles
SCALE = DH**-0.5
W3 = 3 * IC  # 1536: steady-state ACTIVATE width

F32 = mybir.dt.float32
F16 = mybir.dt.float16

DEBUG_DUMP = False


def emit_kernel(ctx, tc, xt, wq, wk, wv, eye, oT):
    nc = tc.nc
    Exp = mybir.ActivationFunctionType.Exp
    mult = mybir.AluOpType.mult
    add = mybir.AluOpType.add

    wpool = ctx.enter_context(tc.tile_pool(name="w", bufs=1))
    qkpool = ctx.enter_context(tc.tile_pool(name="qk", bufs=1))
    ptpool = ctx.enter_context(tc.tile_pool(name="pt", bufs=1))
    npool = ctx.enter_context(tc.tile_pool(name="norm", bufs=2))

    # --- persistent SBUF tensors ---
    ones_sb = wpool.tile([P, DH], F16, tag="ones")
    eye_sb = wpool.tile([DH, DH], F16, tag="eye")
    wq_sb = wpool.tile([P, KT * 256], F16, tag="wq")
    wk_sb = wpool.tile([P, KT * 128], F16, tag="wk")  # [wk|wk] duplicated
    wv_sb = wpool.tile([P, KT * DH], F16, tag="wv")
    xts = wpool.tile([P, KT * N], F16, tag="xt")  # 4MB, [p, (ic, kt, c)]
    qT = [qkpool.tile([P, N], F16, name=f"qT{hp}", tag=f"qT{hp}") for hp in range(2)]
    kkT = qkpool.tile([P, N], F16, tag="kkT")  # kT on both partition halves
    vT_sb = qkpool.tile([DH, N], F16, tag="vT")
    vaug = qkpool.tile([P, NJ * (DH + 1)], F16, tag="vaug")
    nc.any.memset(vaug[:], 1.0)
    nc.any.memset(ones_sb[:], 1.0)
    warm = qkpool.tile([1, 2], F32, tag="warm")
    nc.any.memset(warm[0:1, 0:1], 0.0)
    nc.scalar.activation(warm[0:1, 1:2], warm[0:1, 0:1], Exp)
    # ramp-phase PV accumulators in SBUF: chunk ic0 of each head
    o_sb = [
        npool.tile([DH + 1, IC], F32, name=f"osb{h}", tag=f"osb{h}", bufs=1)
        for h in range(G)
    ]

    # --- input DMAs: each queue runs ~150GB/s, so order each ring's FIFO
    # by when the data is needed; wq's hp0 half rides the otherwise-idle
    # Scalar hwdge queue (done before the exp stream starts) ---
    XW = KT * IC  # columns per ic-chunk of xts

    def dma_x(ic):
        for h in range(2):
            eng = nc.sync if h == 0 else nc.gpsimd
            c0 = ic * XW + h * (XW // 2)
            eng.dma_start(xts[:, c0 : c0 + XW // 2], xt[:, c0 : c0 + XW // 2])

    # The gpsimd SWDGE ring moves only ~60GB/s vs ~150 for the Sync/Scalar
    # HWDGE rings, so everything ramp-critical goes on the two HWDGE rings
    # (Scalar is free until the first exp at ~15us); gpsimd only carries
    # later x halves that have plenty of deadline slack.
    nc.sync.dma_start(wk_sb[:], wk[:, :])
    XQ = XW // 4
    nc.scalar.dma_start(
        wq_sb[:].rearrange("p (k c) -> p k c", k=KT)[:, :, 0:128],
        wq[:, :].rearrange("p (k c) -> p k c", k=KT)[:, :, 0:128],
    )
    for q in (0, 1):
        nc.sync.dma_start(xts[:, q * XQ : (q + 1) * XQ], xt[:, q * XQ : (q + 1) * XQ])
    for q in (2, 3):
        nc.scalar.dma_start(xts[:, q * XQ : (q + 1) * XQ], xt[:, q * XQ : (q + 1) * XQ])
    nc.scalar.dma_start(wv_sb[:], wv[:, :])
    nc.sync.dma_start(eye_sb[:], eye[:, :])
    nc.scalar.dma_start(
        wq_sb[:].rearrange("p (k c) -> p k c", k=KT)[:, :, 128:256],
        wq[:, :].rearrange("p (k c) -> p k c", k=KT)[:, :, 128:256],
    )

    # kt order interleaves the two HWDGE rings' quarter arrival order
    KT_ORDER = [4, 5, 0, 1, 6, 7, 2, 3]

    # --- projection chains (psum pool passed per phase) ---
    def q_chain(hp, ic, pp):
        csl = slice(ic * IC, (ic + 1) * IC)
        ps = pp.tile([P, IC], F32, tag="pj", name="pjq", bufs=2)
        for i, kt in enumerate(KT_ORDER):
            c0 = kt * 256 + hp * 128
            nc.tensor.matmul(
                ps[:],
                wq_sb[:, c0 : c0 + 128],
                xts[:, (ic * KT + kt) * IC : (ic * KT + kt + 1) * IC],
                start=(i == 0),
                stop=(i == KT - 1),
            )
        nc.vector.tensor_copy(qT[hp][:, csl], ps[:])

    def k_mms(ic, pp):
        csl = slice(ic * IC, (ic + 1) * IC)
        ps_k = pp.tile([P, IC], F32, tag="pj", name="pjk", bufs=2)
        for i, kt in enumerate(KT_ORDER):
            nc.tensor.matmul(
                ps_k[:, :],
                wk_sb[:, kt * 128 : (kt + 1) * 128],
                xts[:, (ic * KT + kt) * IC : (ic * KT + kt + 1) * IC],
                start=(i == 0),
                stop=(i == KT - 1),
            )
        nc.vector.tensor_copy(kkT[:, csl], ps_k[:, :])

    def v_mms(ic, pp):
        csl = slice(ic * IC, (ic + 1) * IC)
        ps_v = pp.tile([P, IC], F32, tag="pj", name="pjv", bufs=2)
        for i, kt in enumerate(KT_ORDER):
            nc.tensor.matmul(
                ps_v[0:DH, :],
                wv_sb[:, kt * DH : (kt + 1) * DH],
                xts[:, (ic * KT + kt) * IC : (ic * KT + kt + 1) * IC],
                start=(i == 0),
                stop=(i == KT - 1),
            )
        nc.vector.tensor_copy(vT_sb[:, csl], ps_v[0:DH, :])

    def kv_tail(ic, pp):
        # vaug tiles (v transposed, the ones-row kept from the memset)
        for jt in range(4 * ic, 4 * ic + 4):
            pv = pp.tile([P, DH], F16, tag="pj", bufs=2, name="pvt")
            nc.tensor.transpose(pv[:], vT_sb[:, jt * P : (jt + 1) * P], eye_sb[:])
            nc.vector.tensor_copy(
                vaug[:, jt * (DH + 1) : jt * (DH + 1) + DH], pv[:]
            )

    # --- attention primitives ---
    def qk_pair(hp, ic, jt, dst0, dst1):
        # two row-tiled (concurrent) QK matmuls; head-even scores -> dst0,
        # head-odd -> dst1 (each a [128, 512] psum slice)
        csl = slice(ic * IC, (ic + 1) * IC)
        for half, dst in ((0, dst0), (1, dst1)):
            rsl = slice(half * 64, half * 64 + 64)
            nc.tensor.matmul(
                dst,
                kkT[rsl, jt * P : (jt + 1) * P],
                qT[hp][rsl, csl],
                start=True,
                stop=True,
                tile_position=(half * 64, 0),
            )

    def pv_one(o_ps, jt, mv0, mv1, start, stop):
        for half, mv in ((0, mv0), (1, mv1)):
            nc.tensor.matmul(
                o_ps[half][:],
                vaug[:, jt * (DH + 1) : (jt + 1) * (DH + 1)],
                mv,
                start=start,
                stop=stop,
            )

    def normalize_half(h, ic, fo):
        # GpSimd broadcasts the sums row across partitions (PE-free), then
        # a single-pass approx reciprocal (~18 bits) and the final multiply.
        # partition_broadcast needs its source on partition 0 (HW reads
        # channel 0 regardless of the AP base), so stage the row first.
        csl = slice(ic * IC, (ic + 1) * IC)
        srow = npool.tile([1, IC], F32, tag="srow", bufs=4)
        nc.vector.tensor_copy(srow[:], fo[DH : DH + 1, :])
        bc = npool.tile([DH, IC], F32, name="bcg", tag="bcg", bufs=4)
        nc.gpsimd.partition_broadcast(bc[:], srow[:])
        rb = npool.tile([DH, IC], F32, tag="rb", bufs=4)
        nc.vector.reciprocal_approx_fast(rb[:], bc[:])
        fin = npool.tile([DH, IC], F32, tag="fin", bufs=4)
        nc.vector.tensor_tensor(fin[:], fo[0:DH, :], rb[:], mult)
        nc.sync.dma_start(oT[h * DH : (h + 1) * DH, csl], fin[:])

    # ================= ramp phase =================
    # Both hp-blocks of query chunk 0 run against each key chunk as it is
    # projected: 8 exps per projected key chunk keeps ScalarE nearly fed
    # while the PE also runs the projection chains. PV accumulates per
    # 4-jt group in a 2-bank psum rotation, then adds into o_sb.
    with tc.tile_pool(name="apR", bufs=2, space="PSUM") as stR, tc.tile_pool(
        name="pp", bufs=2, space="PSUM"
    ) as pp, tc.tile_pool(name="pvg", bufs=1, space="PSUM") as pvgp:

        # HAM warmup: near-100% PE duty via N=512 moving operands. The data
        # must be NONZERO -- the activity monitor watches switching, and
        # multiplying zeros generates none.
        warm_mv = wpool.tile([P, IC], F16, tag="warmmv")
        nc.any.memset(warm_mv[:], 1.0)
        for _ in range(8):
            wt = stR.tile([DH, IC], F32, tag="s", name="wt", bufs=2)
            nc.tensor.matmul(
                wt[:], ones_sb[:, 0:DH], warm_mv[:], start=True, stop=True
            )

        def qk4(hp, j0):
            pts = []
            for jt in range(j0, j0 + 4):
                st = stR.tile([P, 2 * IC], F32, tag="s", bufs=2, name="st")
                qk_pair(hp, 0, jt, st[:, 0:IC], st[:, IC : 2 * IC])
                pt = ptpool.tile([P, 2 * IC], F16, tag="pt", bufs=8)
                nc.scalar.activation(pt[:], st[:], Exp, scale=SCALE)
                pts.append(pt)
            return pts

        def pvg4(b, j0, pts):
            for half in range(2):
                pg = pvgp.tile([P, IC], F32, tag=f"pv{half}", bufs=1, name="pg")
                for jt, pt in zip(range(j0, j0 + 4), pts):
                    nc.tensor.matmul(
                        pg[0 : DH + 1, :],
                        vaug[:, jt * (DH + 1) : (jt + 1) * (DH + 1)],
                        pt[:, half * IC : (half + 1) * IC],
                        start=(jt == j0),
                        stop=(jt == j0 + 3),
                    )
                dst = o_sb[2 * b + half]
                if j0 == 0:
                    nc.vector.tensor_copy(dst[:], pg[0 : DH + 1, :])
                else:
                    nc.vector.tensor_tensor(
                        dst[:], dst[:], pg[0 : DH + 1, :], add
                    )

        k_mms(0, pp)
        q_chain(0, 0, pp)
        dma_x(1)
        p0 = qk4(0, 0)
        q_chain(1, 0, pp)
        p1 = qk4(1, 0)
        v_mms(0, pp)
        kv_tail(0, pp)
        pvg4(0, 0, p0)
        pvg4(1, 0, p1)
        for g in range(1, 4):
            k_mms(g, pp)
            p0 = qk4(0, 4 * g)
            if g < 3:
                dma_x(g + 1)
            p1 = qk4(1, 4 * g)
            v_mms(g, pp)
            kv_tail(g, pp)
            pvg4(0, 4 * g, p0)
            pvg4(1, 4 * g, p1)
        # q(0,1) rides the ramp tail so the first steady block can start
        q_chain(0, 1, pp)
        for h in range(G):
            normalize_half(h, 0, o_sb[h])

    # ================= steady phase =================
    # 1536-wide ACTIVATEs: 3 jt of scores fill exactly two [128,1536] psum
    # tiles; PV accumulates a whole block in PSUM as before.
    with tc.tile_pool(name="apS", bufs=2, space="PSUM") as stS, tc.tile_pool(
        name="opS", bufs=1, space="PSUM"
    ) as opool:

        def act3(hp, ic, j0):
            # jts j0, j0+1, j0+2 -> two 1536-wide exp tiles; returns the
            # six [128,512] pt slices in (jt, half) order
            sA = stS.tile([P, W3], F32, tag="s3", bufs=2, name="sA")
            sB = stS.tile([P, W3], F32, tag="s3", bufs=2, name="sB")
            tA = ptpool.tile([P, W3], F16, tag="pt3", bufs=4, name="tA")
            tB = ptpool.tile([P, W3], F16, tag="pt3", bufs=4, name="tB")
            qk_pair(hp, ic, j0, sA[:, 0:IC], sA[:, IC : 2 * IC])
            qk_pair(hp, ic, j0 + 1, sA[:, 2 * IC : W3], sB[:, 0:IC])
            nc.scalar.activation(tA[:], sA[:], Exp, scale=SCALE)
            qk_pair(hp, ic, j0 + 2, sB[:, IC : 2 * IC], sB[:, 2 * IC : W3])
            nc.scalar.activation(tB[:], sB[:], Exp, scale=SCALE)
            return [
                (tA[:, 0:IC], tA[:, IC : 2 * IC]),
                (tA[:, 2 * IC : W3], tB[:, 0:IC]),
                (tB[:, IC : 2 * IC], tB[:, 2 * IC : W3]),
            ]

        def act1(hp, ic, jt):
            st = stS.tile([P, W3], F32, tag="s3", bufs=2, name="s1")
            pt = ptpool.tile([P, W3], F16, tag="pt3", bufs=4, name="t1")
            qk_pair(hp, ic, jt, st[:, 0:IC], st[:, IC : 2 * IC])
            nc.scalar.activation(pt[:, 0 : 2 * IC], st[:, 0 : 2 * IC], Exp, scale=SCALE)
            return [(pt[:, 0:IC], pt[:, IC : 2 * IC])]

        def drain_block(hp, ic, o_ps, last=False):
            # copy out of PSUM promptly so the next block's PV can start;
            # the final block normalizes straight from PSUM
            for half in range(2):
                if last:
                    normalize_half(2 * hp + half, ic, o_ps[half])
                else:
                    fo = npool.tile(
                        [DH + 1, IC], F32, tag="fo", bufs=2, name="fo"
                    )
                    nc.vector.tensor_copy(fo[:], o_ps[half][:])
                    normalize_half(2 * hp + half, ic, fo)

        # remaining q chains, one per steady block, borrowing an s3 slot
        # briefly (all their inputs are resident, so no FIFO stall)
        QCHAINS = {(0, 1): (1, 1), (1, 1): (0, 2), (0, 2): (1, 2),
                   (1, 2): (0, 3), (0, 3): (1, 3)}

        def q_chain_s3(hp, ic):
            csl = slice(ic * IC, (ic + 1) * IC)
            ps = stS.tile([P, W3], F32, tag="s3", bufs=2, name="pjq3")
            for k, kt in enumerate(KT_ORDER):
                c0 = kt * 256 + hp * 128
                nc.tensor.matmul(
                    ps[:, 0:IC],
                    wq_sb[:, c0 : c0 + 128],
                    xts[:, (ic * KT + kt) * IC : (ic * KT + kt + 1) * IC],
                    start=(k == 0),
                    stop=(k == KT - 1),
                )
            nc.vector.tensor_copy(qT[hp][:, csl], ps[:, 0:IC])

        # Flat act-group stream across all steady blocks; PV lags one group
        # so a PV waiting on an ACT never head-blocks the PE FIFO, and the
        # QK/ACT stream never pauses at block boundaries.
        blocks = [(hp, ic) for ic in range(1, NI) for hp in range(2)]
        stream = []
        for bi in range(len(blocks)):
            stream.extend((bi, j0, True) for j0 in (0, 3, 6, 9, 12))
            stream.append((bi, NJ - 1, False))
        o_ps_of = {}
        pend = None

        def flush_pend():
            nonlocal pend
            if pend is None:
                return
            bi, j0, mvs = pend
            hp, ic = blocks[bi]
            if bi not in o_ps_of:
                o_ps_of[bi] = [
                    opool.tile(
                        [DH + 1, IC], F32, name=f"ops{i}", tag=f"ops{i}", bufs=1
                    )
                    for i in range(2)
                ]
            o_ps = o_ps_of[bi]
            for k, (mv0, mv1) in enumerate(mvs):
                jt = j0 + k
                pv_one(o_ps, jt, mv0, mv1, jt == 0, jt == NJ - 1)
            if j0 + len(mvs) - 1 == NJ - 1:
                drain_block(hp, ic, o_ps, last=(bi == len(blocks) - 1))
            pend = None

        for bi, j0, is3 in stream:
            hp, ic = blocks[bi]
            mvs = act3(hp, ic, j0) if is3 else act1(hp, ic, j0)
            if is3 and j0 == 6:
                qc = QCHAINS.pop((hp, ic), None)
                if qc is not None:
                    q_chain_s3(*qc)
            flush_pend()
            pend = (bi, j0, mvs)
        flush_pend()

    if DEBUG_DUMP:
        for name, t, shape in [
            ("dbg_wq", wq_sb, (P, KT * 256)), ("dbg_wk", wk_sb, (P, KT * 128)),
            ("dbg_wv", wv_sb, (P, KT * DH)), ("dbg_kkT", kkT, (P, N)),
            ("dbg_qT0", qT[0], (P, N)), ("dbg_qT1", qT[1], (P, N)),
            ("dbg_vT", vT_sb, (DH, N)), ("dbg_vaug", vaug, (P, NJ * (DH + 1))),
            ("dbg_xts", xts, (P, KT * N)),
        ]:
            d = nc.dram_tensor(name, shape, F16, kind="ExternalOutput").ap()
            nc.sync.dma_start(d[:, :], t[:])


_CACHE = {}


def build():
    if "nc" in _CACHE:
        return _CACHE["nc"]
    nc = bacc.Bacc(
        "TRN2", target_bir_lowering=False, debug=False, num_devices=NCORES
    )
    xt = nc.dram_tensor("xt", (P, KT * N), F16, kind="ExternalInput").ap()
    wq = nc.dram_tensor("wq", (P, KT * 256), F16, kind="ExternalInput").ap()
    wk = nc.dram_tensor("wk", (P, KT * 128), F16, kind="ExternalInput").ap()
    wv = nc.dram_tensor("wv", (P, KT * DH), F16, kind="ExternalInput").ap()
    eye = nc.dram_tensor("eye", (DH, DH), F16, kind="ExternalInput").ap()
    oT = nc.dram_tensor("oT", (G * DH, N), F32, kind="ExternalOutput").ap()
    with tile.TileContext(nc) as tc:
        with ExitStack() as ctx:
            emit_kernel(ctx, tc, xt, wq, wk, wv, eye, oT)
    nc.compile()
    _CACHE["nc"] = nc
    return nc


def _tile_kt(w):
    # (1024, C) -> (128, KT*C): row-block kt lands at column block kt
    C = w.shape[1]
    return np.ascontiguousarray(
        w.reshape(KT, P, C).transpose(1, 0, 2).reshape(P, KT * C)
    )


def make_in_maps(x, Wq, Wkv, k_scale):
    x = np.asarray(x, dtype=np.float32)
    Wq = np.asarray(Wq, dtype=np.float32)
    Wkv = np.asarray(Wkv, dtype=np.float32)
    k_scale = np.asarray(k_scale, dtype=np.float32)
    # x[b].T tiled to [p, (ic, kt, c)] so each ic-chunk is one contiguous DMA
    xts = []
    for b in range(B):
        xT = x[b].T.reshape(KT, P, NI, IC)
        xts.append(
            np.ascontiguousarray(xT.transpose(1, 2, 0, 3).reshape(P, KT * N)).astype(
                np.float16
            )
        )
    in_maps = []
    for c in range(NCORES):
        b, kv = divmod(c, KV_HEADS)
        # fold the per-query-head k_scale into Wq: (q*ks)@k^T == q@(k*ks)^T
        wq_c = np.concatenate(
            [
                Wq[:, (kv * G + j) * DH : (kv * G + j + 1) * DH]
                * k_scale[kv * G + j, 0][None, :]
                for j in range(G)
            ],
            axis=1,
        )
        wk_c = Wkv[:, kv * DH : (kv + 1) * DH]
        wv_c = Wkv[:, KV_HEADS * DH + kv * DH : KV_HEADS * DH + (kv + 1) * DH]
        in_maps.append(
            {
                "xt": xts[b],
                "wq": _tile_kt(wq_c).astype(np.float16),
                "wk": _tile_kt(np.concatenate([wk_c, wk_c], axis=1)).astype(
                    np.float16
                ),
                "wv": _tile_kt(wv_c).astype(np.float16),
                "eye": np.eye(DH, dtype=np.float16),
            }
        )
    return in_maps


def gather(results):
    out = np.empty((B, N, HEADS * DH), dtype=np.float32)
    for c in range(NCORES):
        b, kv = divmod(c, KV_HEADS)
        out[b, :, kv * G * DH : (kv + 1) * G * DH] = results[c]["oT"].T
    return out


def kernel(x, Wq, Wkv, k_scale, _trace=False):
    nc = build()
    in_maps = make_in_maps(x, Wq, Wkv, k_scale)
    res = run_bass_kernel_spmd(
        nc, in_maps, core_ids=list(range(NCORES)), trace=_trace
    )
    out = gather(res.results)
    if _trace:
        kernel.last_result = res
    return out


# revision 60
# speedup vs baseline: 1.1692x; 1.1692x over previous
"""Trainium2 Bass kernel for grouped-query attention with qk-norm.

Problem (hardcoded): x(2,2048,1024) @ Wq(1024,1024) / Wkv(1024,512),
16 query heads, 4 kv heads, head_dim 64, k_scale(16,1,64) applied to the
group-broadcast k. Output (2,2048,1024).

Sharding: 8 cores = batch(2) x kv_heads(4). Each core computes its batch's
4 query heads against its kv head over the full 2048x2048 score matrix.

Device kernel layout choices:
- Host passes x transposed and kt-tiled (dim on partitions) and all
  weights pre-tiled to [128, kt-major], so every input DMA moves
  contiguous 1-8KB lines and projections need no on-device transposes.
- k_scale is folded into Wq host-side ((q*ks)@k^T == q@(k*ks)^T); k is
  projected once per kv head with a host-duplicated [wk|wk] stationary
  (M=128 costs the same as M=64) so PSUM holds kT on both partition
  halves and one copy fills both QK row-tile stationary slots.
- Scores are computed transposed (S^T: keys on partitions, queries free)
  so exp(S^T) feeds the PV matmul directly as the moving operand. All
  matmul inputs are fp16 (fp32r streams at half rate).
- Softmax skips the max-subtraction (inputs bounded) and normalizes
  after PV via an appended ones-row in the V stationary (row 64 of the
  PV psum accumulates sum(exp)).
- The ScalarE exp stream is the pipeline limiter (~1ns/col + 352-cycle
  per-instruction overhead), so: the ramp runs BOTH hp-blocks of the
  first query chunk against each key chunk as it is projected (8 exps
  per projected chunk, PV group-accumulated into SBUF to fit PSUM), and
  the steady phase batches exp into 1536-wide ACTIVATEs (3 jt per 2
  ACTs) with PV accumulating in PSUM.
- Output is returned transposed per head (oT: 4*64 x 2048); the host
  transposes during the gather.
"""

from contextlib import ExitStack

import numpy as np

import concourse.bacc as bacc
import concourse.mybir as mybir
import concourse.tile as tile
from concourse.bass_utils import run_bass_kernel_spmd

# Problem constants
B, N, DIM = 2, 2048, 1024
HEADS, KV_HEADS, DH = 16, 4, 64
G = HEADS // KV_HEADS  # query heads per kv head (4)
NCORES = 8
P = 128
KT = DIM // P  # 8 contraction tiles over dim
IC = 512  # query-chunk width
NI = N // IC  # 4
NJ = N // P  # 16 key tiles
SCALE = DH**-0.5
W3 = 3 * IC  # 1536: steady-state ACTIVATE width

F32 = mybir.dt.float32
F16 = mybir.dt.float16

DEBUG_DUMP = False


def emit_kernel(ctx, tc, xt, wq, wk, wv, eye, oT):
    nc = tc.nc
    Exp = mybir.ActivationFunctionType.Exp
    mult = mybir.AluOpType.mult
    add = mybir.AluOpType.add

    wpool = ctx.enter_context(tc.tile_pool(name="w", bufs=1))
    qkpool = ctx.enter_context(tc.tile_pool(name="qk", bufs=1))
    ptpool = ctx.enter_context(tc.tile_pool(name="pt", bufs=1))
    npool = ctx.enter_context(tc.tile_pool(name="norm", bufs=2))

    # --- persistent SBUF tensors ---
    ones_sb = wpool.tile([P, DH], F16, tag="ones")
    eye_sb = wpool.tile([DH, DH], F16, tag="eye")
    wq_sb = wpool.tile([P, KT * 256], F16, tag="wq")
    wk_sb = wpool.tile([P, KT * 128], F16, tag="wk")  # [wk|wk] duplicated
    wv_sb = wpool.tile([P, KT * DH], F16, tag="wv")
    xts = wpool.tile([P, KT * N], F16, tag="xt")  # 4MB, [p, (ic, kt, c)]
    qT = [qkpool.tile([P, N], F16, name=f"qT{hp}", tag=f"qT{hp}") for hp in range(2)]
    kkT = qkpool.tile([P, N], F16, tag="kkT")  # kT on both partition halves
    vT_sb = qkpool.tile([DH, N], F16, tag="vT")
    vaug = qkpool.tile([P, NJ * (DH + 1)], F16, tag="vaug")
    nc.any.memset(vaug[:], 1.0)
    nc.any.memset(ones_sb[:], 1.0)
    warm = qkpool.tile([1, 2], F32, tag="warm")
    nc.any.memset(warm[0:1, 0:1], 0.0)
    nc.scalar.activation(warm[0:1, 1:2], warm[0:1, 0:1], Exp)
    # ramp-phase PV accumulators in SBUF: chunk ic0 of each head
    o_sb = [
        npool.tile([DH + 1, IC], F32, name=f"osb{h}", tag=f"osb{h}", bufs=1)
        for h in range(G)
    ]

    # --- input DMAs: each queue runs ~150GB/s, so order each ring's FIFO
    # by when the data is needed; wq's hp0 half rides the otherwise-idle
    # Scalar hwdge queue (done before the exp stream starts) ---
    XW = KT * IC  # columns per ic-chunk of xts

    def dma_x(ic):
        for h in range(2):
            eng = nc.sync if h == 0 else nc.gpsimd
            c0 = ic * XW + h * (XW // 2)
            eng.dma_start(xts[:, c0 : c0 + XW // 2], xt[:, c0 : c0 + XW // 2])

    # The gpsimd SWDGE ring moves only ~60GB/s vs ~150 for the Sync/Scalar
    # HWDGE rings, so everything ramp-critical rides the two HWDGE rings
    # (Scalar is free until the first exp); gpsimd only carries later x
    # halves that have deadline slack.
    nc.sync.dma_start(wk_sb[:], wk[:, :])
    XQ = XW // 4
    nc.scalar.dma_start(
        wq_sb[:].rearrange("p (k c) -> p k c", k=KT)[:, :, 0:128],
        wq[:, :].rearrange("p (k c) -> p k c", k=KT)[:, :, 0:128],
    )
    for q in (0, 1):
        nc.sync.dma_start(xts[:, q * XQ : (q + 1) * XQ], xt[:, q * XQ : (q + 1) * XQ])
    for q in (2, 3):
        nc.scalar.dma_start(xts[:, q * XQ : (q + 1) * XQ], xt[:, q * XQ : (q + 1) * XQ])
    nc.scalar.dma_start(wv_sb[:], wv[:, :])
    nc.sync.dma_start(eye_sb[:], eye[:, :])
    nc.scalar.dma_start(
        wq_sb[:].rearrange("p (k c) -> p k c", k=KT)[:, :, 128:256],
        wq[:, :].rearrange("p (k c) -> p k c", k=KT)[:, :, 128:256],
    )

    # kt order interleaves the two HWDGE rings' quarter arrival order
    KT_ORDER = [4, 5, 0, 1, 6, 7, 2, 3]

    # --- projection chains (psum pool passed per phase) ---
    def q_chain(hp, ic, pp):
        csl = slice(ic * IC, (ic + 1) * IC)
        ps = pp.tile([P, IC], F32, tag="pj", name="pjq", bufs=2)
        for i, kt in enumerate(KT_ORDER):
            c0 = kt * 256 + hp * 128
            nc.tensor.matmul(
                ps[:],
                wq_sb[:, c0 : c0 + 128],
                xts[:, (ic * KT + kt) * IC : (ic * KT + kt + 1) * IC],
                start=(i == 0),
                stop=(i == KT - 1),
            )
        nc.vector.tensor_copy(qT[hp][:, csl], ps[:])

    def k_mms(ic, pp):
        csl = slice(ic * IC, (ic + 1) * IC)
        ps_k = pp.tile([P, IC], F32, tag="pj", name="pjk", bufs=2)
        for i, kt in enumerate(KT_ORDER):
            nc.tensor.matmul(
                ps_k[:, :],
                wk_sb[:, kt * 128 : (kt + 1) * 128],
                xts[:, (ic * KT + kt) * IC : (ic * KT + kt + 1) * IC],
                start=(i == 0),
                stop=(i == KT - 1),
            )
        nc.vector.tensor_copy(kkT[:, csl], ps_k[:, :])

    def v_mms(ic, pp):
        csl = slice(ic * IC, (ic + 1) * IC)
        ps_v = pp.tile([P, IC], F32, tag="pj", name="pjv", bufs=2)
        for i, kt in enumerate(KT_ORDER):
            nc.tensor.matmul(
                ps_v[0:DH, :],
                wv_sb[:, kt * DH : (kt + 1) * DH],
                xts[:, (ic * KT + kt) * IC : (ic * KT + kt + 1) * IC],
                start=(i == 0),
                stop=(i == KT - 1),
            )
        nc.vector.tensor_copy(vT_sb[:, csl], ps_v[0:DH, :])

    def kv_tail(ic, pp):
        # vaug tiles (v transposed, the ones-row kept from the memset)
        for jt in range(4 * ic, 4 * ic + 4):
            pv = pp.tile([P, DH], F16, tag="pj", bufs=2, name="pvt")
            nc.tensor.transpose(pv[:], vT_sb[:, jt * P : (jt + 1) * P], eye_sb[:])
            nc.vector.tensor_copy(
                vaug[:, jt * (DH + 1) : jt * (DH + 1) + DH], pv[:]
            )

    # --- attention primitives ---
    def qk_pair(hp, ic, jt, dst0, dst1):
        # two row-tiled (concurrent) QK matmuls; head-even scores -> dst0,
        # head-odd -> dst1 (each a [128, 512] psum slice)
        csl = slice(ic * IC, (ic + 1) * IC)
        for half, dst in ((0, dst0), (1, dst1)):
            rsl = slice(half * 64, half * 64 + 64)
            nc.tensor.matmul(
                dst,
                kkT[rsl, jt * P : (jt + 1) * P],
                qT[hp][rsl, csl],
                start=True,
                stop=True,
                tile_position=(half * 64, 0),
            )

    def pv_one(o_ps, jt, mv0, mv1, start, stop):
        for half, mv in ((0, mv0), (1, mv1)):
            nc.tensor.matmul(
                o_ps[half][:],
                vaug[:, jt * (DH + 1) : (jt + 1) * (DH + 1)],
                mv,
                start=start,
                stop=stop,
            )

    def normalize_half(h, ic, fo):
        # GpSimd broadcasts the sums row across partitions (PE-free), then
        # a single-pass approx reciprocal (~18 bits) and the final multiply.
        # partition_broadcast needs its source on partition 0 (HW reads
        # channel 0 regardless of the AP base), so stage the row first.
        csl = slice(ic * IC, (ic + 1) * IC)
        srow = npool.tile([1, IC], F32, tag="srow", bufs=4)
        nc.vector.tensor_copy(srow[:], fo[DH : DH + 1, :])
        bc = npool.tile([DH, IC], F32, name="bcg", tag="bcg", bufs=4)
        nc.gpsimd.partition_broadcast(bc[:], srow[:])
        rb = npool.tile([DH, IC], F32, tag="rb", bufs=4)
        nc.vector.reciprocal_approx_fast(rb[:], bc[:])
        fin = npool.tile([DH, IC], F32, tag="fin", bufs=4)
        nc.vector.tensor_tensor(fin[:], fo[0:DH, :], rb[:], mult)
        nc.sync.dma_start(oT[h * DH : (h + 1) * DH, csl], fin[:])

    # ================= ramp phase =================
    # Both hp-blocks of query chunk 0 run against each key chunk as it is
    # projected: 8 exps per projected key chunk keeps ScalarE nearly fed
    # while the PE also runs the projection chains. PV accumulates per
    # 4-jt group in a 2-bank psum rotation, then adds into o_sb.
    with tc.tile_pool(name="apR", bufs=2, space="PSUM") as stR, tc.tile_pool(
        name="pp", bufs=2, space="PSUM"
    ) as pp, tc.tile_pool(name="pvg", bufs=1, space="PSUM") as pvgp:

        # HAM warmup: near-100% PE duty via N=512 moving operands. The data
        # must be NONZERO -- the activity monitor watches switching, and
        # multiplying zeros generates none.
        warm_mv = wpool.tile([P, IC], F16, tag="warmmv")
        nc.any.memset(warm_mv[:], 1.0)
        for _ in range(8):
            wt = stR.tile([DH, IC], F32, tag="s", name="wt", bufs=2)
            nc.tensor.matmul(
                wt[:], ones_sb[:, 0:DH], warm_mv[:], start=True, stop=True
            )

        def qk4(hp, j0):
            pts = []
            for jt in range(j0, j0 + 4):
                st = stR.tile([P, 2 * IC], F32, tag="s", bufs=2, name="st")
                qk_pair(hp, 0, jt, st[:, 0:IC], st[:, IC : 2 * IC])
                pt = ptpool.tile([P, 2 * IC], F16, tag="pt", bufs=8)
                nc.scalar.activation(pt[:], st[:], Exp, scale=SCALE)
                pts.append(pt)
            return pts

        def pvg4(b, j0, pts):
            for half in range(2):
                pg = pvgp.tile([P, IC], F32, tag=f"pv{half}", bufs=1, name="pg")
                for jt, pt in zip(range(j0, j0 + 4), pts):
                    nc.tensor.matmul(
                        pg[0 : DH + 1, :],
                        vaug[:, jt * (DH + 1) : (jt + 1) * (DH + 1)],
                        pt[:, half * IC : (half + 1) * IC],
                        start=(jt == j0),
                        stop=(jt == j0 + 3),
                    )
                dst = o_sb[2 * b + half]
                if j0 == 0:
                    nc.vector.tensor_copy(dst[:], pg[0 : DH + 1, :])
                else:
                    nc.vector.tensor_tensor(
                        dst[:], dst[:], pg[0 : DH + 1, :], add
                    )

        k_mms(0, pp)
        q_chain(0, 0, pp)
        dma_x(1)
        p0 = qk4(0, 0)
        q_chain(1, 0, pp)
        p1 = qk4(1, 0)
        v_mms(0, pp)
        kv_tail(0, pp)
        pvg4(0, 0, p0)
        pvg4(1, 0, p1)
        for g in range(1, 4):
            k_mms(g, pp)
            p0 = qk4(0, 4 * g)
            if g < 3:
                dma_x(g + 1)
            p1 = qk4(1, 4 * g)
            v_mms(g, pp)
            kv_tail(g, pp)
            pvg4(0, 4 * g, p0)
            pvg4(1, 4 * g, p1)
        # q(0,1) rides the ramp tail so the first steady block can start
        q_chain(0, 1, pp)
        for h in range(G):
            normalize_half(h, 0, o_sb[h])

    # ================= steady phase =================
    # 1536-wide ACTIVATEs: 3 jt of scores fill exactly two [128,1536] psum
    # tiles; PV accumulates a whole block in PSUM as before.
    with tc.tile_pool(name="apS", bufs=2, space="PSUM") as stS, tc.tile_pool(
        name="opS", bufs=1, space="PSUM"
    ) as opool:

        def act3(hp, ic, j0):
            # jts j0, j0+1, j0+2 -> two 1536-wide exp tiles; returns the
            # six [128,512] pt slices in (jt, half) order
            sA = stS.tile([P, W3], F32, tag="s3", bufs=2, name="sA")
            sB = stS.tile([P, W3], F32, tag="s3", bufs=2, name="sB")
            tA = ptpool.tile([P, W3], F16, tag="pt3", bufs=4, name="tA")
            tB = ptpool.tile([P, W3], F16, tag="pt3", bufs=4, name="tB")
            qk_pair(hp, ic, j0, sA[:, 0:IC], sA[:, IC : 2 * IC])
            qk_pair(hp, ic, j0 + 1, sA[:, 2 * IC : W3], sB[:, 0:IC])
            nc.scalar.activation(tA[:], sA[:], Exp, scale=SCALE)
            qk_pair(hp, ic, j0 + 2, sB[:, IC : 2 * IC], sB[:, 2 * IC : W3])
            nc.scalar.activation(tB[:], sB[:], Exp, scale=SCALE)
            return [
                (tA[:, 0:IC], tA[:, IC : 2 * IC]),
                (tA[:, 2 * IC : W3], tB[:, 0:IC]),
                (tB[:, IC : 2 * IC], tB[:, 2 * IC : W3]),
            ]

        def act1(hp, ic, jt):
            st = stS.tile([P, W3], F32, tag="s3", bufs=2, name="s1")
            pt = ptpool.tile([P, W3], F16, tag="pt3", bufs=4, name="t1")
            qk_pair(hp, ic, jt, st[:, 0:IC], st[:, IC : 2 * IC])
            nc.scalar.activation(pt[:, 0 : 2 * IC], st[:, 0 : 2 * IC], Exp, scale=SCALE)
            return [(pt[:, 0:IC], pt[:, IC : 2 * IC])]

        def drain_block(hp, ic, o_ps, last=False):
            # copy out of PSUM promptly so the next block's PV can start;
            # the final block normalizes straight from PSUM
            for half in range(2):
                if last:
                    normalize_half(2 * hp + half, ic, o_ps[half])
                else:
                    fo = npool.tile(
                        [DH + 1, IC], F32, tag="fo", bufs=2, name="fo"
                    )
                    nc.vector.tensor_copy(fo[:], o_ps[half][:])
                    normalize_half(2 * hp + half, ic, fo)

        # remaining q chains, one per steady block, borrowing an s3 slot
        # briefly (all their inputs are resident, so no FIFO stall)
        QCHAINS = {(0, 1): (1, 1), (1, 1): (0, 2), (0, 2): (1, 2),
                   (1, 2): (0, 3), (0, 3): (1, 3)}

        def q_chain_s3(hp, ic):
            csl = slice(ic * IC, (ic + 1) * IC)
            ps = stS.tile([P, W3], F32, tag="s3", bufs=2, name="pjq3")
            for k, kt in enumerate(KT_ORDER):
                c0 = kt * 256 + hp * 128
                nc.tensor.matmul(
                    ps[:, 0:IC],
                    wq_sb[:, c0 : c0 + 128],
                    xts[:, (ic * KT + kt) * IC : (ic * KT + kt + 1) * IC],
                    start=(k == 0),
                    stop=(k == KT - 1),
                )
            nc.vector.tensor_copy(qT[hp][:, csl], ps[:, 0:IC])

        # Flat act-group stream across all steady blocks; PV lags one group
        # so a PV waiting on an ACT never head-blocks the PE FIFO, and the
        # QK/ACT stream never pauses at block boundaries.
        blocks = [(hp, ic) for ic in range(1, NI) for hp in range(2)]
        stream = []
        for bi in range(len(blocks)):
            stream.extend((bi, j0, True) for j0 in (0, 3, 6, 9, 12))
            stream.append((bi, NJ - 1, False))
        o_ps_of = {}
        pend = None

        def flush_pend():
            nonlocal pend
            if pend is None:
                return
            bi, j0, mvs = pend
            hp, ic = blocks[bi]
            if bi not in o_ps_of:
                o_ps_of[bi] = [
                    opool.tile(
                        [DH + 1, IC], F32, name=f"ops{i}", tag=f"ops{i}", bufs=1
                    )
                    for i in range(2)
                ]
            o_ps = o_ps_of[bi]
            for k, (mv0, mv1) in enumerate(mvs):
                jt = j0 + k
                pv_one(o_ps, jt, mv0, mv1, jt == 0, jt == NJ - 1)
            if j0 + len(mvs) - 1 == NJ - 1:
                drain_block(hp, ic, o_ps, last=(bi == len(blocks) - 1))
            pend = None

        for bi, j0, is3 in stream:
            hp, ic = blocks[bi]
            mvs = act3(hp, ic, j0) if is3 else act1(hp, ic, j0)
            if is3 and j0 == 6:
                qc = QCHAINS.pop((hp, ic), None)
                if qc is not None:
                    q_chain_s3(*qc)
            flush_pend()
            pend = (bi, j0, mvs)
        flush_pend()

    if DEBUG_DUMP:
        for name, t, shape in [
            ("dbg_wq", wq_sb, (P, KT * 256)), ("dbg_wk", wk_sb, (P, KT * 128)),
            ("dbg_wv", wv_sb, (P, KT * DH)), ("dbg_kkT", kkT, (P, N)),
            ("dbg_qT0", qT[0], (P, N)), ("dbg_qT1", qT[1], (P, N)),
            ("dbg_vT", vT_sb, (DH, N)), ("dbg_vaug", vaug, (P, NJ * (DH + 1))),
            ("dbg_xts", xts, (P, KT * N)),
        ]:
            d = nc.dram_tensor(name, shape, F16, kind="ExternalOutput").ap()
            nc.sync.dma_start(d[:, :], t[:])


_CACHE = {}


def build():
    if "nc" in _CACHE:
        return _CACHE["nc"]
    nc = bacc.Bacc(
        "TRN2", target_bir_lowering=False, debug=False, num_devices=NCORES
    )
    xt = nc.dram_tensor("xt", (P, KT * N), F16, kind="ExternalInput").ap()
    wq = nc.dram_tensor("wq", (P, KT * 256), F16, kind="ExternalInput").ap()
    wk = nc.dram_tensor("wk", (P, KT * 128), F16, kind="ExternalInput").ap()
    wv = nc.dram_tensor("wv", (P, KT * DH), F16, kind="ExternalInput").ap()
    eye = nc.dram_tensor("eye", (DH, DH), F16, kind="ExternalInput").ap()
    oT = nc.dram_tensor("oT", (G * DH, N), F32, kind="ExternalOutput").ap()
    with tile.TileContext(nc) as tc:
        with ExitStack() as ctx:
            emit_kernel(ctx, tc, xt, wq, wk, wv, eye, oT)
    nc.compile()
    _CACHE["nc"] = nc
    return nc


def _tile_kt(w):
    # (1024, C) -> (128, KT*C): row-block kt lands at column block kt
    C = w.shape[1]
    return np.ascontiguousarray(
        w.reshape(KT, P, C).transpose(1, 0, 2).reshape(P, KT * C)
    )


def make_in_maps(x, Wq, Wkv, k_scale):
    x = np.asarray(x, dtype=np.float32)
    Wq = np.asarray(Wq, dtype=np.float32)
    Wkv = np.asarray(Wkv, dtype=np.float32)
    k_scale = np.asarray(k_scale, dtype=np.float32)
    # x[b].T tiled to [p, (ic, kt, c)] so each ic-chunk is one contiguous DMA
    xts = []
    for b in range(B):
        xT = x[b].T.reshape(KT, P, NI, IC)
        xts.append(
            np.ascontiguousarray(xT.transpose(1, 2, 0, 3).reshape(P, KT * N)).astype(
                np.float16
            )
        )
    in_maps = []
    for c in range(NCORES):
        b, kv = divmod(c, KV_HEADS)
        # fold the per-query-head k_scale into Wq: (q*ks)@k^T == q@(k*ks)^T
        wq_c = np.concatenate(
            [
                Wq[:, (kv * G + j) * DH : (kv * G + j + 1) * DH]
                * k_scale[kv * G + j, 0][None, :]
                for j in range(G)
            ],
            axis=1,
        )
        wk_c = Wkv[:, kv * DH : (kv + 1) * DH]
        wv_c = Wkv[:, KV_HEADS * DH + kv * DH : KV_HEADS * DH + (kv + 1) * DH]
        in_maps.append(
            {
                "xt": xts[b],
                "wq": _tile_kt(wq_c).astype(np.float16),
                "wk": _tile_kt(np.concatenate([wk_c, wk_c], axis=1)).astype(
                    np.float16
                ),
                "wv": _tile_kt(wv_c).astype(np.float16),
                "eye": np.eye(DH, dtype=np.float16),
            }
        )
    return in_maps


def gather(results):
    out = np.empty((B, N, HEADS * DH), dtype=np.float32)
    for c in range(NCORES):
        b, kv = divmod(c, KV_HEADS)
        out[b, :, kv * G * DH : (kv + 1) * G * DH] = results[c]["oT"].T
    return out


def kernel(x, Wq, Wkv, k_scale, _trace=False):
    nc = build()
    in_maps = make_in_maps(x, Wq, Wkv, k_scale)
    res = run_bass_kernel_spmd(
        nc, in_maps, core_ids=list(range(NCORES)), trace=_trace
    )
    out = gather(res.results)
    if _trace:
        kernel.last_result = res
    return out
